# revision 27
# baseline (speedup 1.0000x reference)
"""Trainium2 Bass kernel for an RNN-T style joint network MLP.

  out[b,t,u,o] = tanh(enc[b,t,:] @ W1[:512] + dec[b,u,:] @ W1[512:] + b1) @ W2 + b2

Shapes: enc (8, 256, 512), dec (8, 64, 512), W1 (1024, 1024), b1 (1024,),
W2 (1024, 128), b2 (128,), out (8, 256, 64, 128), all float32.

Sharding: data-parallel over batch - one batch element per NeuronCore, no
collectives. Per core the kernel is elementwise-bound: ACT does 16.8M tanh
at 1 elem/cycle/lane (109us floor) and DVE does the 512 bias-broadcast
adds sum[h,u,t] = e_proj[h,t] + bias[h,u] (tensor_scalar 2x mode, ~196ns
per [128,256], 100us) plus most of the PSUM evacuation; the steady state
balances both at ~117us by giving ACT the pair-0 evacuation on odd blocks.
Measured: ~148.6us HW exec (8 axon trn2 cores), rel err 4.2e-3. The head
is HBM-read-bound (8 cores x 2.6MB of replicated inputs at ~65GB/s per
queue) and the steady state is jointly ACT+DVE saturated, so the remaining
gap to the ~128us analytic floor is pipeline ramp and drain.
PE (main GEMM, bf16) and DMA ride far below. Key scheduling devices:
  - 2-block-lagged consumer emission so DVE never queues behind a
    not-yet-runnable PSUM evac;
  - ~70 junk warmup matmuls spanning the load phase so the HAM clock-gate
    is at 2.4GHz when the real GEMMs arrive;
  - a dummy tanh at t=0 to hoist the ~2.7us ACT_TABLE_LOAD;
  - W1 host-packed as 16 contiguous 128KB half-chunks (hc x enc/dec side)
    spread over both HW DGE queues in first-use order; block 0's tanh is
    eighth-split per h-chunk so ACT starts as soon as the first chunk
    lands; blocks 1-2 are quarter/half-split while DVE ramps;
  - the last block's tanh is split per-u (strided AP) with per-u GEMM,
    evac and DMA chasing the drain across both queues;
  - output is bf16 in [U, O, T] layout (contiguous writes; host upcasts,
    ~1e-3 extra error against the 2e-2 budget).
"""

import os
import numpy as np
import ml_dtypes

B, T, U, D, H, O = 8, 256, 64, 512, 1024, 128
NCORES = 8
HC = H // 128      # 8 h-chunks
UB = 4             # u-block size
NB = U // UB       # 16 blocks
BW = UB * T * HC   # 8192 per-block sum/tanh width (hc-major: [hc][u][t])
WARM_MMS = 78      # junk matmuls spanning the load phase (HAM stays warm)
ACT_EVAC_BLKS = {1, 3, 5, 7, 9, 10, 11, 13}  # pair-0 evac on ACT

_CACHE = {}
LAST_RESULT = None


def _build_program():
    from concourse import bacc, tile
    import concourse.mybir as mybir

    dt = mybir.dt
    f32, bf16 = dt.float32, dt.bfloat16
    Act = mybir.ActivationFunctionType

    nc = bacc.Bacc("TRN2", target_bir_lowering=False, debug=False)

    encT = nc.dram_tensor("encT", [128, 4 * T], bf16, kind="ExternalInput").ap()
    decT = nc.dram_tensor("decT", [128, 4 * U], bf16, kind="ExternalInput").ap()
    w1r = nc.dram_tensor("w1r", [HC, 2, 128, 512], bf16,
                         kind="ExternalInput").ap()
    w2r = nc.dram_tensor("w2r", [128, HC * O], bf16, kind="ExternalInput").ap()
    b1r = nc.dram_tensor("b1r", [128, HC], f32, kind="ExternalInput").ap()
    b2c = nc.dram_tensor("b2c", [O, 1], f32, kind="ExternalInput").ap()
    outT = nc.dram_tensor("outT", [U, O, T], bf16, kind="ExternalOutput").ap()

    with tile.TileContext(nc) as tc:
        with tc.tile_pool(name="persist", bufs=1) as persist, \
             tc.tile_pool(name="sums", bufs=4) as sums_pool, \
             tc.tile_pool(name="tanhp", bufs=4) as tanh_pool, \
             tc.tile_pool(name="outsb", bufs=6) as out_pool, \
             tc.tile_pool(name="psum", bufs=8, space="PSUM") as psum_pool:

            w1_sb = persist.tile([128, HC * 1024], bf16, tag="w1")
            encT_sb = persist.tile([128, 4 * T], bf16, tag="encT")
            decT_sb = persist.tile([128, 4 * U], bf16, tag="decT")
            w2_sb = persist.tile([128, HC * O], bf16, tag="w2")
            b1_sb = persist.tile([128, HC], f32, tag="b1")
            b2_sb = persist.tile([128, 1], f32, tag="b2")
            e_sb = persist.tile([128, HC * T], bf16, tag="eproj")
            bias_sb = persist.tile([128, HC * U], f32, tag="bias")
            junk_w = persist.tile([128, 128], bf16, tag="junkw")
            junk_r = persist.tile([128, 128], bf16, tag="junkr")
            warm_a = persist.tile([128, 128], bf16, tag="warma")
            warm_o = persist.tile([128, 128], bf16, tag="warmo")

            def w1_dst(hc, side):
                lo = hc * 1024 + side * 512
                return w1_sb[:, lo:lo + 512]

            # --- input DMAs on the two HW DGE queues, first-use order ----
            # (each w1r[hc, side] is a contiguous 128KB read)
            nc.scalar.dma_start(decT_sb[:], decT[:, :])
            nc.scalar.dma_start(w1_dst(0, 1), w1r[0, 1, :, :])
            nc.scalar.dma_start(w1_dst(0, 0), w1r[0, 0, :, :])
            for hc in (2, 4, 6):
                nc.scalar.dma_start(
                    w1_sb[:, hc * 1024:(hc + 1) * 1024]
                    .rearrange("p (s x) -> p s x", s=2),
                    w1r[hc, :, :, :].rearrange("s p x -> p s x"))
            nc.sync.dma_start(b1_sb[:], b1r[:, :])
            nc.sync.dma_start(encT_sb[:, :2 * T], encT[:, :2 * T])
            nc.sync.dma_start(encT_sb[:, 2 * T:], encT[:, 2 * T:])
            for hc in (1, 3, 5, 7):
                nc.sync.dma_start(w1_dst(hc, 0), w1r[hc, 0, :, :])
                nc.sync.dma_start(w1_dst(hc, 1), w1r[hc, 1, :, :])
            # gpsimd software DGE carries the non-critical tail loads
            nc.gpsimd.dma_start(w2_sb[:], w2r[:, :])
            nc.gpsimd.dma_start(b2_sb[:], b2c[:, :])

            # --- t=0 warmups ---------------------------------------------
            nc.vector.memset(junk_w[:], 0.0)
            nc.vector.memset(junk_r[:], 0.0)
            nc.vector.memset(warm_a[:], 0.0)
            nc.scalar.activation(warm_o[:], warm_a[:], Act.Tanh)
            for i in range(WARM_MMS):
                jp = psum_pool.tile([128, 128], f32, tag="ps", name=f"junk{i}")
                nc.tensor.matmul(jp[:], lhsT=junk_w[:], rhs=junk_r[:],
                                 start=True, stop=True)

            # --- head: first GEMMs per h-chunk, fused with block-0 adds --
            sum0 = sums_pool.tile([128, BW], bf16, tag="sum")
            tanh0 = tanh_pool.tile([128, BW], bf16, tag="tanh")
            for hc in range(HC):
                pd = psum_pool.tile([128, U], f32, tag="ps", name=f"pd{hc}")
                for dc in range(4):
                    nc.tensor.matmul(
                        pd[:],
                        lhsT=w1_sb[:, hc * 1024 + 512 + dc * 128:
                                   hc * 1024 + 512 + dc * 128 + 128],
                        rhs=decT_sb[:, dc * U:(dc + 1) * U],
                        start=(dc == 0), stop=(dc == 3),
                    )
                # bias evac on ACT: it slots into the DMA-pacing gaps
                # between eighth-tanhs, and keeps DVE's ramp deficit small
                # (DVE is what the early blocks wait on)
                nc.scalar.activation(bias_sb[:, hc * U:(hc + 1) * U], pd[:],
                                     Act.Identity, bias=b1_sb[:, hc:hc + 1])

                pe = psum_pool.tile([128, T], f32, tag="ps", name=f"pe{hc}")
                for dc in range(4):
                    nc.tensor.matmul(
                        pe[:],
                        lhsT=w1_sb[:, hc * 1024 + dc * 128:
                                   hc * 1024 + dc * 128 + 128],
                        rhs=encT_sb[:, dc * T:(dc + 1) * T],
                        start=(dc == 0), stop=(dc == 3),
                    )
                nc.vector.tensor_copy(e_sb[:, hc * T:(hc + 1) * T], pe[:])

                # block-0 adds for this h-chunk, then its eighth of the
                # block-0 tanh (gates on just this h-chunk)
                for ul in range(UB):
                    nc.vector.tensor_scalar_add(
                        sum0[:, hc * (UB * T) + ul * T:
                             hc * (UB * T) + ul * T + T],
                        e_sb[:, hc * T:(hc + 1) * T],
                        bias_sb[:, hc * U + ul: hc * U + ul + 1])
                nc.scalar.activation(
                    tanh0[:, hc * (UB * T):(hc + 1) * (UB * T)],
                    sum0[:, hc * (UB * T):(hc + 1) * (UB * T)], Act.Tanh)

            # --- steady loop, software-pipelined emission (lag 2) --------
            tanh_tiles = {0: tanh0}

            def emit_consumer(blk):
                """main GEMM + evac + out DMA for block blk."""
                tanh_sb = tanh_tiles.pop(blk)
                if blk == NB - 1:
                    # per-u chase: 4 psum tiles, u-outer GEMM, per-u
                    # evac + DMA split across both queues
                    for s in range(UB):
                        ps = psum_pool.tile([128, T], f32, tag="ps",
                                            name=f"pl{s}")
                        for hc in range(HC):
                            nc.tensor.matmul(
                                ps[:],
                                lhsT=w2_sb[:, hc * O:(hc + 1) * O],
                                rhs=tanh_sb[:, hc * (UB * T) + s * T:
                                            hc * (UB * T) + (s + 1) * T],
                                start=(hc == 0), stop=(hc == HC - 1),
                            )
                        osb = out_pool.tile([128, T], bf16, tag="osb",
                                            name=f"ot{s}")
                        # alternate the drain evacs across ACT (idle after
                        # the last tanh) and DVE so they don't serialize
                        if s % 2 == 0:
                            nc.scalar.activation(osb[:], ps[:], Act.Identity,
                                                 bias=b2_sb[:, 0:1])
                        else:
                            nc.vector.tensor_scalar_add(osb[:], ps[:],
                                                        b2_sb[:, 0:1])
                        eng = nc.scalar if s % 2 == 0 else nc.sync
                        eng.dma_start(outT[blk * UB + s, :, :], osb[:])
                    return
                pos = [psum_pool.tile([128, 512], f32, tag="ps",
                                      name=f"po{blk}_{p}") for p in range(2)]
                for hc in range(HC):
                    for p in range(2):
                        nc.tensor.matmul(
                            pos[p][:],
                            lhsT=w2_sb[:, hc * O:(hc + 1) * O],
                            rhs=tanh_sb[:, hc * (UB * T) + p * 512:
                                        hc * (UB * T) + (p + 1) * 512],
                            start=(hc == 0), stop=(hc == HC - 1),
                        )
                for p in range(2):
                    osb = out_pool.tile([128, 512], bf16, tag="osb",
                                        name=f"o{blk}_{p}")
                    if p == 0 and blk in ACT_EVAC_BLKS:
                        nc.scalar.activation(osb[:], pos[p][:], Act.Identity,
                                             bias=b2_sb[:, 0:1])
                    else:
                        nc.vector.tensor_scalar_add(osb[:], pos[p][:],
                                                    b2_sb[:, 0:1])
                    u0 = blk * UB + 2 * p
                    nc.sync.dma_start(
                        outT[u0:u0 + 2, :, :].rearrange("u o t -> o u t"),
                        osb[:, :].rearrange("p (u t) -> p u t", u=2))

            for blk in range(1, NB):
                sum_sb = sums_pool.tile([128, BW], bf16, tag="sum")
                for hc in range(HC):
                    for ul in range(UB):
                        u = blk * UB + ul
                        nc.vector.tensor_scalar_add(
                            sum_sb[:, hc * (UB * T) + ul * T:
                                   hc * (UB * T) + ul * T + T],
                            e_sb[:, hc * T:(hc + 1) * T],
                            bias_sb[:, hc * U + u: hc * U + u + 1])

                tanh_sb = tanh_pool.tile([128, BW], bf16, tag="tanh")
                if blk in (1, 2):
                    for q in range(4):
                        nc.scalar.activation(
                            tanh_sb[:, q * BW // 4:(q + 1) * BW // 4],
                            sum_sb[:, q * BW // 4:(q + 1) * BW // 4],
                            Act.Tanh)
                elif blk in (3, 4, 5, 6):
                    for hq in range(2):
                        nc.scalar.activation(
                            tanh_sb[:, hq * BW // 2:(hq + 1) * BW // 2],
                            sum_sb[:, hq * BW // 2:(hq + 1) * BW // 2],
                            Act.Tanh)
                elif blk == NB - 1:
                    # per-u split (strided [128, hc=8, 256] APs)
                    sv = sum_sb[:, :].rearrange("p (c x) -> p c x", c=HC)
                    tv = tanh_sb[:, :].rearrange("p (c x) -> p c x", c=HC)
                    for s in range(UB):
                        nc.scalar.activation(
                            tv[:, :, s * T:(s + 1) * T],
                            sv[:, :, s * T:(s + 1) * T], Act.Tanh)
                else:
                    nc.scalar.activation(tanh_sb[:], sum_sb[:], Act.Tanh)
                tanh_tiles[blk] = tanh_sb

                # emit the block-(k-2) consumer: its PSUM results are
                # already final, so DVE/ACT never stall on these
                if blk >= 2:
                    emit_consumer(blk - 2)

            emit_consumer(NB - 2)
            emit_consumer(NB - 1)

    nc.compile()
    return nc


def kernel(encoder_state, decoder_state, W1, b1, W2, b2):
    from concourse.bass_utils import run_bass_kernel_spmd
    global LAST_RESULT

    if "nc" not in _CACHE:
        _CACHE["nc"] = _build_program()
    nc = _CACHE["nc"]

    encoder_state = np.asarray(encoder_state, dtype=np.float32)
    decoder_state = np.asarray(decoder_state, dtype=np.float32)
    W1 = np.asarray(W1, dtype=np.float32)
    b1 = np.asarray(b1, dtype=np.float32)
    W2 = np.asarray(W2, dtype=np.float32)
    b2 = np.asarray(b2, dtype=np.float32)

    bf = ml_dtypes.bfloat16
    # W1 [2D, H] -> [hc, side(enc/dec), 128, dc*128]: each [128, 512]
    # half-chunk is contiguous (128KB); d-within-chunk on partitions.
    W1r = np.ascontiguousarray(
        W1.astype(bf).reshape(2, 4, 128, 8, 128).transpose(3, 0, 2, 1, 4)
        .reshape(HC, 2, 128, 512))
    # W2 [H, O] -> [128, hc(8)*O] with h-within-chunk on partitions
    W2r = np.ascontiguousarray(
        W2.astype(bf).reshape(8, 128, O).transpose(1, 0, 2).reshape(128, 8 * O))
    b1r = np.ascontiguousarray(b1.reshape(HC, 128).T)
    b2c = np.ascontiguousarray(b2.reshape(O, 1))

    in_maps = []
    for i in range(NCORES):
        encTa = np.ascontiguousarray(
            encoder_state[i].T.astype(bf).reshape(4, 128, T)
            .transpose(1, 0, 2).reshape(128, 4 * T))
        decTa = np.ascontiguousarray(
            decoder_state[i].T.astype(bf).reshape(4, 128, U)
            .transpose(1, 0, 2).reshape(128, 4 * U))
        in_maps.append({
            "encT": encTa,
            "decT": decTa,
            "w1r": W1r,
            "w2r": W2r,
            "b1r": b1r,
            "b2c": b2c,
        })

    trace = bool(int(os.environ.get("KERNEL_TRACE", "0")))
    res = run_bass_kernel_spmd(nc, in_maps, list(range(NCORES)), trace=trace)
    LAST_RESULT = res

    out = np.empty((B, T, U, O), dtype=np.float32)
    for i in range(NCORES):
        # outT [U, O, T] -> out[b, t, u, o]
        out[i] = res.results[i]["outT"].astype(np.float32).transpose(2, 0, 1)
    return out


# revision 30
# speedup vs baseline: 1.1952x; 1.1952x over previous
"""Trainium2 Bass kernel for an RNN-T style joint network MLP.

  out[b,t,u,o] = tanh(enc[b,t,:] @ W1[:512] + dec[b,u,:] @ W1[512:] + b1) @ W2 + b2

Shapes: enc (8, 256, 512), dec (8, 64, 512), W1 (1024, 1024), b1 (1024,),
W2 (1024, 128), b2 (128,), out (8, 256, 64, 128), all float32.

Sharding: data-parallel over batch - one batch element per NeuronCore, no
collectives. Per core the kernel is elementwise-bound: ACT does 16.8M tanh
at 1 elem/cycle/lane (109us floor) and DVE does the 512 bias-broadcast
adds sum[h,u,t] = e_proj[h,t] + bias[h,u] (tensor_scalar 2x mode, ~196ns
per [128,256], 100us) plus most of the PSUM evacuation; the steady state
balances both at ~117us by giving ACT the pair-0 evacuation on odd blocks.
Measured: ~148.6us HW exec (8 axon trn2 cores), rel err 4.2e-3. The head
is HBM-read-bound (8 cores x 2.6MB of replicated inputs at ~65GB/s per
queue) and the steady state is jointly ACT+DVE saturated, so the remaining
gap to the ~128us analytic floor is pipeline ramp and drain.
PE (main GEMM, bf16) and DMA ride far below. Key scheduling devices:
  - 2-block-lagged consumer emission so DVE never queues behind a
    not-yet-runnable PSUM evac;
  - ~70 junk warmup matmuls spanning the load phase so the HAM clock-gate
    is at 2.4GHz when the real GEMMs arrive;
  - a dummy tanh at t=0 to hoist the ~2.7us ACT_TABLE_LOAD;
  - W1 host-packed as 16 contiguous 128KB half-chunks (hc x enc/dec side)
    spread over both HW DGE queues in first-use order; block 0's tanh is
    eighth-split per h-chunk so ACT starts as soon as the first chunk
    lands; blocks 1-2 are quarter/half-split while DVE ramps;
  - the last block's tanh is split per-u (strided AP) with per-u GEMM,
    evac and DMA chasing the drain across both queues;
  - output is bf16 in [U, O, T] layout (contiguous writes; host upcasts,
    ~1e-3 extra error against the 2e-2 budget).
"""

import os
import numpy as np
import ml_dtypes

B, T, U, D, H, O = 8, 256, 64, 512, 1024, 128
NCORES = 8
HC = H // 128      # 8 h-chunks
UB = 4             # u-block size
NB = U // UB       # 16 blocks
BW = UB * T * HC   # 8192 per-block sum/tanh width (hc-major: [hc][u][t])
WARM_MMS = 70      # junk matmuls spanning the load phase (HAM stays warm)
ACT_EVAC_BLKS = {1, 3, 5, 7, 9, 11, 13}  # pair-0 evac on ACT

_CACHE = {}
LAST_RESULT = None


def _build_program():
    from concourse import bacc, tile
    import concourse.mybir as mybir

    dt = mybir.dt
    f32, bf16 = dt.float32, dt.bfloat16
    Act = mybir.ActivationFunctionType

    nc = bacc.Bacc("TRN2", target_bir_lowering=False, debug=False)

    encT = nc.dram_tensor("encT", [128, 4 * T], bf16, kind="ExternalInput").ap()
    decT = nc.dram_tensor("decT", [128, 4 * U], bf16, kind="ExternalInput").ap()
    w1r = nc.dram_tensor("w1r", [HC, 2, 128, 512], bf16,
                         kind="ExternalInput").ap()
    w2r = nc.dram_tensor("w2r", [128, HC * O], bf16, kind="ExternalInput").ap()
    b1r = nc.dram_tensor("b1r", [128, HC], f32, kind="ExternalInput").ap()
    b2c = nc.dram_tensor("b2c", [O, 1], f32, kind="ExternalInput").ap()
    outT = nc.dram_tensor("outT", [U, O, T], bf16, kind="ExternalOutput").ap()

    with tile.TileContext(nc) as tc:
        with tc.tile_pool(name="persist", bufs=1) as persist, \
             tc.tile_pool(name="sums", bufs=4) as sums_pool, \
             tc.tile_pool(name="tanhp", bufs=4) as tanh_pool, \
             tc.tile_pool(name="outsb", bufs=6) as out_pool, \
             tc.tile_pool(name="psum", bufs=8, space="PSUM") as psum_pool:

            w1_sb = persist.tile([128, HC * 1024], bf16, tag="w1")
            encT_sb = persist.tile([128, 4 * T], bf16, tag="encT")
            decT_sb = persist.tile([128, 4 * U], bf16, tag="decT")
            w2_sb = persist.tile([128, HC * O], bf16, tag="w2")
            b1_sb = persist.tile([128, HC], f32, tag="b1")
            b2_sb = persist.tile([128, 1], f32, tag="b2")
            e_sb = persist.tile([128, HC * T], bf16, tag="eproj")
            bias_sb = persist.tile([128, HC * U], f32, tag="bias")
            junk_w = persist.tile([128, 128], bf16, tag="junkw")
            junk_r = persist.tile([128, 128], bf16, tag="junkr")
            warm_a = persist.tile([128, 128], bf16, tag="warma")
            warm_o = persist.tile([128, 128], bf16, tag="warmo")

            def w1_dst(hc, side):
                lo = hc * 1024 + side * 512
                return w1_sb[:, lo:lo + 512]

            # --- input DMAs on the two HW DGE queues, first-use order ----
            # (each w1r[hc, side] is a contiguous 128KB read)
            nc.scalar.dma_start(decT_sb[:], decT[:, :])
            nc.scalar.dma_start(w1_dst(0, 1), w1r[0, 1, :, :])
            nc.scalar.dma_start(w1_dst(0, 0), w1r[0, 0, :, :])
            for hc in (2, 4, 6):
                nc.scalar.dma_start(
                    w1_sb[:, hc * 1024:(hc + 1) * 1024]
                    .rearrange("p (s x) -> p s x", s=2),
                    w1r[hc, :, :, :].rearrange("s p x -> p s x"))
            nc.sync.dma_start(b1_sb[:], b1r[:, :])
            nc.sync.dma_start(encT_sb[:], encT[:, :])
            for hc in (1, 3, 5, 7):
                nc.sync.dma_start(w1_dst(hc, 0), w1r[hc, 0, :, :])
                nc.sync.dma_start(w1_dst(hc, 1), w1r[hc, 1, :, :])
            # gpsimd software DGE carries the non-critical tail loads
            nc.gpsimd.dma_start(w2_sb[:], w2r[:, :])
            nc.gpsimd.dma_start(b2_sb[:], b2c[:, :])

            # --- t=0 warmups ---------------------------------------------
            nc.vector.memset(junk_w[:], 0.0)
            nc.vector.memset(junk_r[:], 0.0)
            nc.vector.memset(warm_a[:], 0.0)
            nc.scalar.activation(warm_o[:], warm_a[:], Act.Tanh)
            for i in range(WARM_MMS):
                jp = psum_pool.tile([128, 128], f32, tag="ps", name=f"junk{i}")
                nc.tensor.matmul(jp[:], lhsT=junk_w[:], rhs=junk_r[:],
                                 start=True, stop=True)

            # --- head: first GEMMs per h-chunk, fused with block-0 adds --
            sum0 = sums_pool.tile([128, BW], bf16, tag="sum")
            tanh0 = tanh_pool.tile([128, BW], bf16, tag="tanh")
            for hc in range(HC):
                pd = psum_pool.tile([128, U], f32, tag="ps", name=f"pd{hc}")
                for dc in range(4):
                    nc.tensor.matmul(
                        pd[:],
                        lhsT=w1_sb[:, hc * 1024 + 512 + dc * 128:
                                   hc * 1024 + 512 + dc * 128 + 128],
                        rhs=decT_sb[:, dc * U:(dc + 1) * U],
                        start=(dc == 0), stop=(dc == 3),
                    )
                # bias evac on ACT: it slots into the DMA-pacing gaps
                # between eighth-tanhs, and keeps DVE's ramp deficit small
                # (DVE is what the early blocks wait on)
                nc.scalar.activation(bias_sb[:, hc * U:(hc + 1) * U], pd[:],
                                     Act.Identity, bias=b1_sb[:, hc:hc + 1])

                pe = psum_pool.tile([128, T], f32, tag="ps", name=f"pe{hc}")
                for dc in range(4):
                    nc.tensor.matmul(
                        pe[:],
                        lhsT=w1_sb[:, hc * 1024 + dc * 128:
                                   hc * 1024 + dc * 128 + 128],
                        rhs=encT_sb[:, dc * T:(dc + 1) * T],
                        start=(dc == 0), stop=(dc == 3),
                    )
                nc.vector.tensor_copy(e_sb[:, hc * T:(hc + 1) * T], pe[:])

                # block-0 adds for this h-chunk, then its eighth of the
                # block-0 tanh (gates on just this h-chunk)
                for ul in range(UB):
                    nc.vector.tensor_scalar_add(
                        sum0[:, hc * (UB * T) + ul * T:
                             hc * (UB * T) + ul * T + T],
                        e_sb[:, hc * T:(hc + 1) * T],
                        bias_sb[:, hc * U + ul: hc * U + ul + 1])
                nc.scalar.activation(
                    tanh0[:, hc * (UB * T):(hc + 1) * (UB * T)],
                    sum0[:, hc * (UB * T):(hc + 1) * (UB * T)], Act.Tanh)

            # --- steady loop, software-pipelined emission (lag 2) --------
            tanh_tiles = {0: tanh0}

            def emit_consumer(blk):
                """main GEMM + evac + out DMA for block blk."""
                tanh_sb = tanh_tiles.pop(blk)
                if blk == NB - 1:
                    # per-u chase: 4 psum tiles, u-outer GEMM, per-u
                    # evac + DMA split across both queues
                    for s in range(UB):
                        ps = psum_pool.tile([128, T], f32, tag="ps",
                                            name=f"pl{s}")
                        for hc in range(HC):
                            nc.tensor.matmul(
                                ps[:],
                                lhsT=w2_sb[:, hc * O:(hc + 1) * O],
                                rhs=tanh_sb[:, hc * (UB * T) + s * T:
                                            hc * (UB * T) + (s + 1) * T],
                                start=(hc == 0), stop=(hc == HC - 1),
                            )
                        osb = out_pool.tile([128, T], bf16, tag="osb",
                                            name=f"ot{s}")
                        # alternate the drain evacs across ACT (idle after
                        # the last tanh) and DVE so they don't serialize
                        if s % 2 == 0:
                            nc.scalar.activation(osb[:], ps[:], Act.Identity,
                                                 bias=b2_sb[:, 0:1])
                        else:
                            nc.vector.tensor_scalar_add(osb[:], ps[:],
                                                        b2_sb[:, 0:1])
                        eng = nc.scalar if s % 2 == 0 else nc.sync
                        eng.dma_start(outT[blk * UB + s, :, :], osb[:])
                    return
                pos = [psum_pool.tile([128, 512], f32, tag="ps",
                                      name=f"po{blk}_{p}") for p in range(2)]
                for hc in range(HC):
                    for p in range(2):
                        nc.tensor.matmul(
                            pos[p][:],
                            lhsT=w2_sb[:, hc * O:(hc + 1) * O],
                            rhs=tanh_sb[:, hc * (UB * T) + p * 512:
                                        hc * (UB * T) + (p + 1) * 512],
                            start=(hc == 0), stop=(hc == HC - 1),
                        )
                for p in range(2):
                    osb = out_pool.tile([128, 512], bf16, tag="osb",
                                        name=f"o{blk}_{p}")
                    if p == 0 and blk in ACT_EVAC_BLKS:
                        nc.scalar.activation(osb[:], pos[p][:], Act.Identity,
                                             bias=b2_sb[:, 0:1])
                    else:
                        nc.vector.tensor_scalar_add(osb[:], pos[p][:],
                                                    b2_sb[:, 0:1])
                    u0 = blk * UB + 2 * p
                    nc.sync.dma_start(
                        outT[u0:u0 + 2, :, :].rearrange("u o t -> o u t"),
                        osb[:, :].rearrange("p (u t) -> p u t", u=2))

            for blk in range(1, NB):
                sum_sb = sums_pool.tile([128, BW], bf16, tag="sum")
                for hc in range(HC):
                    for ul in range(UB):
                        u = blk * UB + ul
                        nc.vector.tensor_scalar_add(
                            sum_sb[:, hc * (UB * T) + ul * T:
                                   hc * (UB * T) + ul * T + T],
                            e_sb[:, hc * T:(hc + 1) * T],
                            bias_sb[:, hc * U + u: hc * U + u + 1])

                tanh_sb = tanh_pool.tile([128, BW], bf16, tag="tanh")
                if blk in (1, 2):
                    for q in range(4):
                        nc.scalar.activation(
                            tanh_sb[:, q * BW // 4:(q + 1) * BW // 4],
                            sum_sb[:, q * BW // 4:(q + 1) * BW // 4],
                            Act.Tanh)
                elif blk in (3, 4, 5, 6):
                    for hq in range(2):
                        nc.scalar.activation(
                            tanh_sb[:, hq * BW // 2:(hq + 1) * BW // 2],
                            sum_sb[:, hq * BW // 2:(hq + 1) * BW // 2],
                            Act.Tanh)
                elif blk == NB - 1:
                    # per-u split (strided [128, hc=8, 256] APs)
                    sv = sum_sb[:, :].rearrange("p (c x) -> p c x", c=HC)
                    tv = tanh_sb[:, :].rearrange("p (c x) -> p c x", c=HC)
                    for s in range(UB):
                        nc.scalar.activation(
                            tv[:, :, s * T:(s + 1) * T],
                            sv[:, :, s * T:(s + 1) * T], Act.Tanh)
                else:
                    nc.scalar.activation(tanh_sb[:], sum_sb[:], Act.Tanh)
                tanh_tiles[blk] = tanh_sb

                # emit the block-(k-2) consumer: its PSUM results are
                # already final, so DVE/ACT never stall on these
                if blk >= 2:
                    emit_consumer(blk - 2)

            emit_consumer(NB - 2)
            emit_consumer(NB - 1)

    nc.compile()
    return nc


def kernel(encoder_state, decoder_state, W1, b1, W2, b2):
    from concourse.bass_utils import run_bass_kernel_spmd
    global LAST_RESULT

    if "nc" not in _CACHE:
        _CACHE["nc"] = _build_program()
    nc = _CACHE["nc"]

    encoder_state = np.asarray(encoder_state, dtype=np.float32)
    decoder_state = np.asarray(decoder_state, dtype=np.float32)
    W1 = np.asarray(W1, dtype=np.float32)
    b1 = np.asarray(b1, dtype=np.float32)
    W2 = np.asarray(W2, dtype=np.float32)
    b2 = np.asarray(b2, dtype=np.float32)

    bf = ml_dtypes.bfloat16
    # W1 [2D, H] -> [hc, side(enc/dec), 128, dc*128]: each [128, 512]
    # half-chunk is contiguous (128KB); d-within-chunk on partitions.
    W1r = np.ascontiguousarray(
        W1.astype(bf).reshape(2, 4, 128, 8, 128).transpose(3, 0, 2, 1, 4)
        .reshape(HC, 2, 128, 512))
    # W2 [H, O] -> [128, hc(8)*O] with h-within-chunk on partitions
    W2r = np.ascontiguousarray(
        W2.astype(bf).reshape(8, 128, O).transpose(1, 0, 2).reshape(128, 8 * O))
    b1r = np.ascontiguousarray(b1.reshape(HC, 128).T)
    b2c = np.ascontiguousarray(b2.reshape(O, 1))

    in_maps = []
    for i in range(NCORES):
        encTa = np.ascontiguousarray(
            encoder_state[i].T.astype(bf).reshape(4, 128, T)
            .transpose(1, 0, 2).reshape(128, 4 * T))
        decTa = np.ascontiguousarray(
            decoder_state[i].T.astype(bf).reshape(4, 128, U)
            .transpose(1, 0, 2).reshape(128, 4 * U))
        in_maps.append({
            "encT": encTa,
            "decT": decTa,
            "w1r": W1r,
            "w2r": W2r,
            "b1r": b1r,
            "b2c": b2c,
        })

    trace = bool(int(os.environ.get("KERNEL_TRACE", "0")))
    res = run_bass_kernel_spmd(nc, in_maps, list(range(NCORES)), trace=trace)
    LAST_RESULT = res

    out = np.empty((B, T, U, O), dtype=np.float32)
    for i in range(NCORES):
        # outT [U, O, T] -> out[b, t, u, o]
        out[i] = res.results[i]["outT"].astype(np.float32).transpose(2, 0, 1)
    return out


# revision 34
# speedup vs baseline: 1.1958x; 1.0005x over previous
"""Trainium2 Bass kernel for an RNN-T style joint network MLP.

  out[b,t,u,o] = tanh(enc[b,t,:] @ W1[:512] + dec[b,u,:] @ W1[512:] + b1) @ W2 + b2

Shapes: enc (8, 256, 512), dec (8, 64, 512), W1 (1024, 1024), b1 (1024,),
W2 (1024, 128), b2 (128,), out (8, 256, 64, 128), all float32.

Sharding: data-parallel over batch - one batch element per NeuronCore, no
collectives. Per core the kernel is elementwise-bound: ACT does 16.8M tanh
at 1 elem/cycle/lane (109us floor) and DVE does the 512 bias-broadcast
adds sum[h,u,t] = e_proj[h,t] + bias[h,u] (tensor_scalar 2x mode, ~196ns
per [128,256], 100us) plus most of the PSUM evacuation; the steady state
balances both at ~117us by giving ACT the pair-0 evacuation on odd blocks.
Measured: ~148.6us HW exec (8 axon trn2 cores), rel err 4.2e-3. The head
is HBM-read-bound (8 cores x 2.6MB of replicated inputs at ~65GB/s per
queue) and the steady state is jointly ACT+DVE saturated, so the remaining
gap to the ~128us analytic floor is pipeline ramp and drain.
PE (main GEMM, bf16) and DMA ride far below. Key scheduling devices:
  - 2-block-lagged consumer emission so DVE never queues behind a
    not-yet-runnable PSUM evac;
  - ~70 junk warmup matmuls spanning the load phase so the HAM clock-gate
    is at 2.4GHz when the real GEMMs arrive;
  - a dummy tanh at t=0 to hoist the ~2.7us ACT_TABLE_LOAD;
  - W1 host-packed as 16 contiguous 128KB half-chunks (hc x enc/dec side)
    spread over both HW DGE queues in first-use order; block 0's tanh is
    eighth-split per h-chunk so ACT starts as soon as the first chunk
    lands; blocks 1-2 are quarter/half-split while DVE ramps;
  - the last block's tanh is split per-u (strided AP) with per-u GEMM,
    evac and DMA chasing the drain across both queues;
  - output is bf16 in [U, O, T] layout (contiguous writes; host upcasts,
    ~1e-3 extra error against the 2e-2 budget).
"""

import os
import numpy as np
import ml_dtypes

B, T, U, D, H, O = 8, 256, 64, 512, 1024, 128
NCORES = 8
HC = H // 128      # 8 h-chunks
UB = 4             # u-block size
NB = U // UB       # 16 blocks
BW = UB * T * HC   # 8192 per-block sum/tanh width (hc-major: [hc][u][t])
WARM_MMS = 70      # junk matmuls spanning the load phase (HAM stays warm)
ACT_EVAC_BLKS = {1, 3, 5, 7, 9, 11, 13, 14}  # pair-0 evac on ACT

_CACHE = {}
LAST_RESULT = None


def _build_program():
    from concourse import bacc, tile
    import concourse.mybir as mybir

    dt = mybir.dt
    f32, bf16 = dt.float32, dt.bfloat16
    Act = mybir.ActivationFunctionType

    nc = bacc.Bacc("TRN2", target_bir_lowering=False, debug=False)

    encT = nc.dram_tensor("encT", [128, 4 * T], bf16, kind="ExternalInput").ap()
    decT = nc.dram_tensor("decT", [128, 4 * U], bf16, kind="ExternalInput").ap()
    w1r = nc.dram_tensor("w1r", [HC, 2, 128, 512], bf16,
                         kind="ExternalInput").ap()
    w2r = nc.dram_tensor("w2r", [128, HC * O], bf16, kind="ExternalInput").ap()
    b1r = nc.dram_tensor("b1r", [128, HC], f32, kind="ExternalInput").ap()
    b2c = nc.dram_tensor("b2c", [O, 1], f32, kind="ExternalInput").ap()
    outT = nc.dram_tensor("outT", [U, O, T], bf16, kind="ExternalOutput").ap()

    with tile.TileContext(nc) as tc:
        with tc.tile_pool(name="persist", bufs=1) as persist, \
             tc.tile_pool(name="sums", bufs=4) as sums_pool, \
             tc.tile_pool(name="tanhp", bufs=4) as tanh_pool, \
             tc.tile_pool(name="outsb", bufs=6) as out_pool, \
             tc.tile_pool(name="psum", bufs=8, space="PSUM") as psum_pool:

            w1_sb = persist.tile([128, HC * 1024], bf16, tag="w1")
            encT_sb = persist.tile([128, 4 * T], bf16, tag="encT")
            decT_sb = persist.tile([128, 4 * U], bf16, tag="decT")
            w2_sb = persist.tile([128, HC * O], bf16, tag="w2")
            b1_sb = persist.tile([128, HC], f32, tag="b1")
            b2_sb = persist.tile([128, 1], f32, tag="b2")
            e_sb = persist.tile([128, HC * T], bf16, tag="eproj")
            bias_sb = persist.tile([128, HC * U], f32, tag="bias")
            junk_w = persist.tile([128, 128], bf16, tag="junkw")
            junk_r = persist.tile([128, 128], bf16, tag="junkr")
            warm_a = persist.tile([128, 128], bf16, tag="warma")
            warm_o = persist.tile([128, 128], bf16, tag="warmo")

            def w1_dst(hc, side):
                lo = hc * 1024 + side * 512
                return w1_sb[:, lo:lo + 512]

            # --- input DMAs on the two HW DGE queues, first-use order ----
            # (each w1r[hc, side] is a contiguous 128KB read)
            nc.scalar.dma_start(decT_sb[:], decT[:, :])
            nc.scalar.dma_start(w1_dst(0, 1), w1r[0, 1, :, :])
            nc.scalar.dma_start(w1_dst(0, 0), w1r[0, 0, :, :])
            for hc in (2, 4, 6):
                nc.scalar.dma_start(
                    w1_sb[:, hc * 1024:(hc + 1) * 1024]
                    .rearrange("p (s x) -> p s x", s=2),
                    w1r[hc, :, :, :].rearrange("s p x -> p s x"))
            nc.sync.dma_start(b1_sb[:], b1r[:, :])
            nc.sync.dma_start(encT_sb[:], encT[:, :])
            for hc in (1, 3, 5, 7):
                nc.sync.dma_start(w1_dst(hc, 0), w1r[hc, 0, :, :])
                nc.sync.dma_start(w1_dst(hc, 1), w1r[hc, 1, :, :])
            # gpsimd software DGE carries the non-critical tail loads
            nc.gpsimd.dma_start(w2_sb[:], w2r[:, :])
            nc.gpsimd.dma_start(b2_sb[:], b2c[:, :])

            # --- t=0 warmups ---------------------------------------------
            nc.vector.memset(junk_w[:], 0.0)
            nc.vector.memset(junk_r[:], 0.0)
            nc.vector.memset(warm_a[:], 0.0)
            nc.scalar.activation(warm_o[:], warm_a[:], Act.Tanh)
            for i in range(WARM_MMS):
                jp = psum_pool.tile([128, 128], f32, tag="ps", name=f"junk{i}")
                nc.tensor.matmul(jp[:], lhsT=junk_w[:], rhs=junk_r[:],
                                 start=True, stop=True)

            # --- head: first GEMMs per h-chunk, fused with block-0 adds --
            sum0 = sums_pool.tile([128, BW], bf16, tag="sum")
            tanh0 = tanh_pool.tile([128, BW], bf16, tag="tanh")
            for hc in range(HC):
                def emit_pd(hc=hc):
                    pd = psum_pool.tile([128, U], f32, tag="ps",
                                        name=f"pd{hc}")
                    for dc in range(4):
                        nc.tensor.matmul(
                            pd[:],
                            lhsT=w1_sb[:, hc * 1024 + 512 + dc * 128:
                                       hc * 1024 + 512 + dc * 128 + 128],
                            rhs=decT_sb[:, dc * U:(dc + 1) * U],
                            start=(dc == 0), stop=(dc == 3),
                        )
                    # bias evac on ACT: slots into the DMA-pacing gaps
                    # between eighth-tanhs; keeps DVE's ramp deficit small
                    nc.scalar.activation(bias_sb[:, hc * U:(hc + 1) * U],
                                         pd[:], Act.Identity,
                                         bias=b1_sb[:, hc:hc + 1])

                def emit_pe(hc=hc):
                    pe = psum_pool.tile([128, T], f32, tag="ps",
                                        name=f"pe{hc}")
                    for dc in range(4):
                        nc.tensor.matmul(
                            pe[:],
                            lhsT=w1_sb[:, hc * 1024 + dc * 128:
                                       hc * 1024 + dc * 128 + 128],
                            rhs=encT_sb[:, dc * T:(dc + 1) * T],
                            start=(dc == 0), stop=(dc == 3),
                        )
                    nc.vector.tensor_copy(e_sb[:, hc * T:(hc + 1) * T],
                                          pe[:])

                # match PE emission order to DMA arrival order: sync-queue
                # chunks (odd hc) land enc-half first; hc0's halves land
                # dec-first; hc2/4/6 arrive atomically
                if hc % 2 == 1:
                    emit_pe()
                    emit_pd()
                else:
                    emit_pd()
                    emit_pe()

                # block-0 adds for this h-chunk, then its eighth of the
                # block-0 tanh (gates on just this h-chunk)
                for ul in range(UB):
                    nc.vector.tensor_scalar_add(
                        sum0[:, hc * (UB * T) + ul * T:
                             hc * (UB * T) + ul * T + T],
                        e_sb[:, hc * T:(hc + 1) * T],
                        bias_sb[:, hc * U + ul: hc * U + ul + 1])
                nc.scalar.activation(
                    tanh0[:, hc * (UB * T):(hc + 1) * (UB * T)],
                    sum0[:, hc * (UB * T):(hc + 1) * (UB * T)], Act.Tanh)

            # --- steady loop, software-pipelined emission (lag 2) --------
            tanh_tiles = {0: tanh0}

            def emit_consumer(blk):
                """main GEMM + evac + out DMA for block blk."""
                tanh_sb = tanh_tiles.pop(blk)
                if blk == NB - 1:
                    # per-u chase: 4 psum tiles, u-outer GEMM, per-u
                    # evac + DMA split across both queues
                    for s in range(UB):
                        ps = psum_pool.tile([128, T], f32, tag="ps",
                                            name=f"pl{s}")
                        for hc in range(HC):
                            nc.tensor.matmul(
                                ps[:],
                                lhsT=w2_sb[:, hc * O:(hc + 1) * O],
                                rhs=tanh_sb[:, hc * (UB * T) + s * T:
                                            hc * (UB * T) + (s + 1) * T],
                                start=(hc == 0), stop=(hc == HC - 1),
                            )
                        osb = out_pool.tile([128, T], bf16, tag="osb",
                                            name=f"ot{s}")
                        # alternate the drain evacs across ACT (idle after
                        # the last tanh) and DVE so they don't serialize;
                        # the final one goes to ACT
                        if s % 2 == 1:
                            nc.scalar.activation(osb[:], ps[:], Act.Identity,
                                                 bias=b2_sb[:, 0:1])
                        else:
                            nc.vector.tensor_scalar_add(osb[:], ps[:],
                                                        b2_sb[:, 0:1])
                        eng = nc.scalar if s % 2 == 1 else nc.sync
                        eng.dma_start(outT[blk * UB + s, :, :], osb[:])
                    return
                pos = [psum_pool.tile([128, 512], f32, tag="ps",
                                      name=f"po{blk}_{p}") for p in range(2)]
                for hc in range(HC):
                    for p in range(2):
                        nc.tensor.matmul(
                            pos[p][:],
                            lhsT=w2_sb[:, hc * O:(hc + 1) * O],
                            rhs=tanh_sb[:, hc * (UB * T) + p * 512:
                                        hc * (UB * T) + (p + 1) * 512],
                            start=(hc == 0), stop=(hc == HC - 1),
                        )
                for p in range(2):
                    osb = out_pool.tile([128, 512], bf16, tag="osb",
                                        name=f"o{blk}_{p}")
                    if p == 0 and blk in ACT_EVAC_BLKS:
                        nc.scalar.activation(osb[:], pos[p][:], Act.Identity,
                                             bias=b2_sb[:, 0:1])
                    else:
                        nc.vector.tensor_scalar_add(osb[:], pos[p][:],
                                                    b2_sb[:, 0:1])
                    u0 = blk * UB + 2 * p
                    nc.sync.dma_start(
                        outT[u0:u0 + 2, :, :].rearrange("u o t -> o u t"),
                        osb[:, :].rearrange("p (u t) -> p u t", u=2))

            for blk in range(1, NB):
                sum_sb = sums_pool.tile([128, BW], bf16, tag="sum")
                for hc in range(HC):
                    for ul in range(UB):
                        u = blk * UB + ul
                        nc.vector.tensor_scalar_add(
                            sum_sb[:, hc * (UB * T) + ul * T:
                                   hc * (UB * T) + ul * T + T],
                            e_sb[:, hc * T:(hc + 1) * T],
                            bias_sb[:, hc * U + u: hc * U + u + 1])

                tanh_sb = tanh_pool.tile([128, BW], bf16, tag="tanh")
                if blk in (1, 2):
                    for q in range(4):
                        nc.scalar.activation(
                            tanh_sb[:, q * BW // 4:(q + 1) * BW // 4],
                            sum_sb[:, q * BW // 4:(q + 1) * BW // 4],
                            Act.Tanh)
                elif blk in (3, 4, 5, 6):
                    for hq in range(2):
                        nc.scalar.activation(
                            tanh_sb[:, hq * BW // 2:(hq + 1) * BW // 2],
                            sum_sb[:, hq * BW // 2:(hq + 1) * BW // 2],
                            Act.Tanh)
                elif blk == NB - 1:
                    # per-u split (strided [128, hc=8, 256] APs)
                    sv = sum_sb[:, :].rearrange("p (c x) -> p c x", c=HC)
                    tv = tanh_sb[:, :].rearrange("p (c x) -> p c x", c=HC)
                    for s in range(UB):
                        nc.scalar.activation(
                            tv[:, :, s * T:(s + 1) * T],
                            sv[:, :, s * T:(s + 1) * T], Act.Tanh)
                else:
                    nc.scalar.activation(tanh_sb[:], sum_sb[:], Act.Tanh)
                tanh_tiles[blk] = tanh_sb

                # emit the block-(k-2) consumer: its PSUM results are
                # already final, so DVE/ACT never stall on these
                if blk >= 2:
                    emit_consumer(blk - 2)

            emit_consumer(NB - 2)
            emit_consumer(NB - 1)

    nc.compile()
    return nc


def kernel(encoder_state, decoder_state, W1, b1, W2, b2):
    from concourse.bass_utils import run_bass_kernel_spmd
    global LAST_RESULT

    if "nc" not in _CACHE:
        _CACHE["nc"] = _build_program()
    nc = _CACHE["nc"]

    encoder_state = np.asarray(encoder_state, dtype=np.float32)
    decoder_state = np.asarray(decoder_state, dtype=np.float32)
    W1 = np.asarray(W1, dtype=np.float32)
    b1 = np.asarray(b1, dtype=np.float32)
    W2 = np.asarray(W2, dtype=np.float32)
    b2 = np.asarray(b2, dtype=np.float32)

    bf = ml_dtypes.bfloat16
    # W1 [2D, H] -> [hc, side(enc/dec), 128, dc*128]: each [128, 512]
    # half-chunk is contiguous (128KB); d-within-chunk on partitions.
    W1r = np.ascontiguousarray(
        W1.astype(bf).reshape(2, 4, 128, 8, 128).transpose(3, 0, 2, 1, 4)
        .reshape(HC, 2, 128, 512))
    # W2 [H, O] -> [128, hc(8)*O] with h-within-chunk on partitions
    W2r = np.ascontiguousarray(
        W2.astype(bf).reshape(8, 128, O).transpose(1, 0, 2).reshape(128, 8 * O))
    b1r = np.ascontiguousarray(b1.reshape(HC, 128).T)
    b2c = np.ascontiguousarray(b2.reshape(O, 1))

    in_maps = []
    for i in range(NCORES):
        encTa = np.ascontiguousarray(
            encoder_state[i].T.astype(bf).reshape(4, 128, T)
            .transpose(1, 0, 2).reshape(128, 4 * T))
        decTa = np.ascontiguousarray(
            decoder_state[i].T.astype(bf).reshape(4, 128, U)
            .transpose(1, 0, 2).reshape(128, 4 * U))
        in_maps.append({
            "encT": encTa,
            "decT": decTa,
            "w1r": W1r,
            "w2r": W2r,
            "b1r": b1r,
            "b2c": b2c,
        })

    trace = bool(int(os.environ.get("KERNEL_TRACE", "0")))
    res = run_bass_kernel_spmd(nc, in_maps, list(range(NCORES)), trace=trace)
    LAST_RESULT = res

    out = np.empty((B, T, U, O), dtype=np.float32)
    for i in range(NCORES):
        # outT [U, O, T] -> out[b, t, u, o]
        out[i] = res.results[i]["outT"].astype(np.float32).transpose(2, 0, 1)
    return out
